# revision 1
# baseline (speedup 1.0000x reference)
"""Trainium2 Bass kernel for the Jordan-model forward pass.

out = sigmoid(tanh(x @ W_x.T + b_h) @ W_out.T + b_out)
  x: [262144, 512] f32, W_hidden: [64, 576] (only first 512 cols used),
  b_hidden: [64], W_out: [64, 64], b_out: [64]  ->  out: [262144, 64] f32

Data parallel over 8 NeuronCores (32768 rows each). Per 512-row block:
  - one 1MB DMA load of x (natural layout)
  - PE transposes x chunks into [d, b] layout (fp32r transpose, exact)
  - mm1 in "transposed" orientation: psum_hT[64h, 512b] accumulated over 4
    K-chunks with stationary W slices (fp32r, N=512 -> full PE rate)
  - ACT tanh with per-partition bias
  - mm2 back to natural orientation: stationary hT column-slices (stride 4)
    so psum partitions hold 4 consecutive output rows -> 1KB-contiguous
    stores at DMA line rate
  - DVE adds broadcast b_out in PSUM, ACT sigmoid, one 128KB DMA store
"""

import sys
from contextlib import ExitStack

sys.path.insert(0, "/opt/trn_rl_repo")

import numpy as np

import concourse.bass as bass
import concourse.mybir as mybir
import concourse.tile as tile
from concourse import bacc
from concourse.bass_utils import run_bass_kernel_spmd
from concourse.masks import make_identity

N_CORES = 8
B = 262144
D = 512
H = 64
O = 64
B_LOCAL = B // N_CORES  # 32768
BLK = 512  # batch rows per block
N_BLKS = B_LOCAL // BLK  # 64
KC = D // 128  # 4 contraction chunks

F32 = mybir.dt.float32
F32R = mybir.dt.float32r
TANH = mybir.ActivationFunctionType.Tanh
SIGMOID = mybir.ActivationFunctionType.Sigmoid


def _r(ap):
    return ap.bitcast(F32R)


def build_kernel():
    nc = bacc.Bacc("TRN2", target_bir_lowering=False, debug=False, num_devices=N_CORES)
    x = nc.dram_tensor("x", [B_LOCAL, D], F32, kind="ExternalInput").ap()
    wh = nc.dram_tensor("w_hidden", [H, D + O], F32, kind="ExternalInput").ap()
    bh = nc.dram_tensor("b_hidden", [H], F32, kind="ExternalInput").ap()
    wo = nc.dram_tensor("w_out", [O, H], F32, kind="ExternalInput").ap()
    bo = nc.dram_tensor("b_out", [O], F32, kind="ExternalInput").ap()
    out = nc.dram_tensor("out", [B_LOCAL, O], F32, kind="ExternalOutput").ap()

    with tile.TileContext(nc) as tc, ExitStack() as ctx:
        const = ctx.enter_context(tc.tile_pool(name="const", bufs=1))
        psetup = ctx.enter_context(tc.tile_pool(name="psetup", bufs=1, space="PSUM"))

        ident = const.tile([128, 128], F32)
        make_identity(nc, ident)

        # ---- weight prep (one-time) ----
        wx_sb = const.tile([H, D], F32)
        nc.gpsimd.dma_start(wx_sb, wh[:, 0:D])
        wxT = const.tile([128, KC, H], F32R)  # chunk k -> wxT[:, k, :] = W_x[:, k].T
        for k in range(KC):
            pt = psetup.tile([128, H], F32, tag="pt")
            nc.tensor.transpose(pt, wx_sb[:, k * 128:(k + 1) * 128],
                                ident[0:H, 0:H])
            nc.scalar.copy(wxT[:, k, :], pt)

        wo_sb = const.tile([O, H], F32)
        nc.gpsimd.dma_start(wo_sb, wo)
        woT = const.tile([H, O], F32R)
        pwo = psetup.tile([H, O], F32, tag="pt")
        nc.tensor.transpose(pwo, wo_sb, ident[0:O, 0:O])
        nc.scalar.copy(woT, pwo)

        bh_sb = const.tile([H, 1], F32)
        nc.gpsimd.dma_start(bh_sb, bh.rearrange("(h one) -> h one", one=1))

        # b_out broadcast to [128, 4, 64] via rank-1 matmul with a ones row
        bo_row = const.tile([1, O], F32)
        nc.gpsimd.dma_start(bo_row, bo.rearrange("(one o) -> one o", one=1))
        ones_row = const.tile([1, 128], F32)
        nc.vector.memset(ones_row, 1.0)
        pbo = psetup.tile([128, O], F32, tag="pt")
        nc.tensor.matmul(pbo, lhsT=ones_row, rhs=bo_row, start=True, stop=True)
        bo4 = const.tile([128, 4, O], F32)
        for t in range(4):
            nc.scalar.copy(bo4[:, t, :], pbo)

        # ---- pipelined main loop ----
        xpool = ctx.enter_context(tc.tile_pool(name="xpool", bufs=3))
        xtpool = ctx.enter_context(tc.tile_pool(name="xtpool", bufs=8))
        hpool = ctx.enter_context(tc.tile_pool(name="hpool", bufs=3))
        opool = ctx.enter_context(tc.tile_pool(name="opool", bufs=3))
        pxt_pool = ctx.enter_context(tc.tile_pool(name="pxt", bufs=2, space="PSUM"))
        ph_pool = ctx.enter_context(tc.tile_pool(name="ph", bufs=2, space="PSUM"))
        po_pool = ctx.enter_context(tc.tile_pool(name="po", bufs=2, space="PSUM"))

        for i in range(N_BLKS):
            b0 = i * BLK
            # load 512 rows as [p, t, d]; row = t*128 + p
            xb = xpool.tile([128, 4, D], F32)
            nc.gpsimd.dma_start(
                xb, x[b0:b0 + BLK, :].rearrange("(t p) d -> p t d", p=128))

            phT = ph_pool.tile([H, BLK], F32)
            for k in range(KC):
                ks = slice(k * 128, (k + 1) * 128)
                pxT = pxt_pool.tile([128, BLK], F32)
                for t in range(4):
                    nc.tensor.transpose(
                        pxT[:, t * 128:(t + 1) * 128], xb[:, t, ks], ident)
                xT = xtpool.tile([128, BLK], F32R)
                if k % 2 == 0:
                    nc.scalar.copy(xT, pxT)
                else:
                    nc.vector.tensor_copy(xT, pxT)
                nc.tensor.matmul(phT, lhsT=wxT[:, k, :], rhs=xT,
                                 start=(k == 0), stop=(k == KC - 1))

            hT = hpool.tile([H, BLK], F32R)
            nc.scalar.activation(hT, phT, TANH, bias=bh_sb[:, 0:1])

            # mm2: out rows 4p+t live in psum partition p, free slice t
            po_t = po_pool.tile([128, 4, O], F32)
            hT4 = hT.rearrange("h (j four) -> h four j", four=4)
            for t in range(4):
                nc.tensor.matmul(po_t[:, t, :], lhsT=hT4[:, t, :],
                                 rhs=woT, start=True, stop=True)

            nc.vector.tensor_add(po_t, po_t, bo4)
            ob = opool.tile([128, 4, O], F32)
            nc.scalar.activation(ob, po_t, SIGMOID)

            nc.gpsimd.dma_start(
                out[b0:b0 + BLK, :].rearrange("(p four) o -> p four o", four=4),
                ob)

    nc.compile()
    return nc


_NC = None


def _get_nc():
    global _NC
    if _NC is None:
        _NC = build_kernel()
    return _NC


def kernel(x, W_hidden, b_hidden, W_out, b_out):
    x = np.ascontiguousarray(x, dtype=np.float32)
    W_hidden = np.ascontiguousarray(W_hidden, dtype=np.float32)
    b_hidden = np.ascontiguousarray(b_hidden, dtype=np.float32)
    W_out = np.ascontiguousarray(W_out, dtype=np.float32)
    b_out = np.ascontiguousarray(b_out, dtype=np.float32)

    nc = _get_nc()
    shards = np.split(x, N_CORES, axis=0)
    in_maps = [{
        "x": shards[c],
        "w_hidden": W_hidden,
        "b_hidden": b_hidden,
        "w_out": W_out,
        "b_out": b_out,
    } for c in range(N_CORES)]
    res = run_bass_kernel_spmd(nc, in_maps, list(range(N_CORES)))
    return np.concatenate([res.results[c]["out"] for c in range(N_CORES)], axis=0)


if __name__ == "__main__":
    rng = np.random.default_rng(0)
    x = rng.standard_normal((B, D), dtype=np.float32)
    wh = (rng.standard_normal((H, D + O), dtype=np.float32) / np.sqrt(D + O))
    bh_ = rng.standard_normal(H, dtype=np.float32) * 0.01
    wo_ = rng.standard_normal((O, H), dtype=np.float32) / np.sqrt(H)
    bo_ = rng.standard_normal(O, dtype=np.float32) * 0.01
    got = kernel(x=x, W_hidden=wh, b_hidden=bh_, W_out=wo_, b_out=bo_)
    hid = np.tanh(x @ wh[:, :D].T + bh_)
    want = 1.0 / (1.0 + np.exp(-(hid @ wo_.T + bo_)))
    err = np.abs(got - want)
    rel = err.max() / np.abs(want).max()
    print(f"max abs err {err.max():.3e}  rel {rel:.3e}")



# revision 2
# speedup vs baseline: 205.7764x; 205.7764x over previous
"""Trainium2 Bass kernel for the Jordan-model forward pass.

out = sigmoid(tanh(x @ W_x.T + b_h) @ W_out.T + b_out)
  x: [262144, 512] f32, W_hidden: [64, 576] (only first 512 cols used),
  b_hidden: [64], W_out: [64, 64], b_out: [64]  ->  out: [262144, 64] f32

Data parallel over 8 NeuronCores (32768 rows each). Per 1024-row block:
  - one 2MB DMA load with a single contiguous 16KiB descriptor per
    partition: xb[p, t, :] = x[b0 + 8p + t] (Pool SWDGE queue)
  - PE transposes x chunks into [d, b] layout (fp32 transpose, exact)
  - mm1 in transposed orientation per 512-row half: psum_hT[64h, 512b]
    accumulated over 4 K-chunks with stationary fp32r W slices (full PE
    rate at 512-wide moving dim)
  - ACT tanh per half with per-partition bias -> bf16 hT
  - mm2 in bf16 (1 cycle/row) with contiguous hT column slices:
    po[:, s, :] holds out rows 8p+s
  - one DVE bias-add + one ACT sigmoid + one 2KiB/partition store per block
  - weight prep and output stores ride the SP HWDGE queue so x-load issue
    on the Pool queue is never blocked
bf16 is only used after tanh (values in [-1,1]); rel err ~2e-3 vs the
2e-2 gate. The kernel is HBM-bandwidth-bound.
"""

import sys
from contextlib import ExitStack

sys.path.insert(0, "/opt/trn_rl_repo")

import numpy as np

import concourse.bass as bass
import concourse.mybir as mybir
import concourse.tile as tile
from concourse import bacc
from concourse.bass_utils import run_bass_kernel_spmd
from concourse.masks import make_identity

N_CORES = 8
B = 262144
D = 512
H = 64
O = 64
B_LOCAL = B // N_CORES  # 32768
BLK = 1024
N_BLKS = B_LOCAL // BLK  # 32
KC = D // 128  # 4 contraction chunks

F32 = mybir.dt.float32
F32R = mybir.dt.float32r
BF16 = mybir.dt.bfloat16
TANH = mybir.ActivationFunctionType.Tanh
SIGMOID = mybir.ActivationFunctionType.Sigmoid


def build_kernel(repeat=1):
    """Build the nc. `repeat` unrolls the whole computation R times inside
    one NEFF — used by test.py to measure true HW time via marginal slope."""
    nc = bacc.Bacc("TRN2", target_bir_lowering=False, debug=False, num_devices=N_CORES)
    x = nc.dram_tensor("x", [B_LOCAL, D], F32, kind="ExternalInput").ap()
    wh = nc.dram_tensor("w_hidden", [H, D + O], F32, kind="ExternalInput").ap()
    bh = nc.dram_tensor("b_hidden", [H], F32, kind="ExternalInput").ap()
    wo = nc.dram_tensor("w_out", [O, H], F32, kind="ExternalInput").ap()
    bo = nc.dram_tensor("b_out", [O], F32, kind="ExternalInput").ap()
    out = nc.dram_tensor("out", [B_LOCAL, O], F32, kind="ExternalOutput").ap()

    with tile.TileContext(nc) as tc, ExitStack() as ctx:
        const = ctx.enter_context(tc.tile_pool(name="const", bufs=1))
        xpool = ctx.enter_context(tc.tile_pool(name="xpool", bufs=4))
        xtpool = ctx.enter_context(tc.tile_pool(name="xtpool", bufs=4))
        hpool = ctx.enter_context(tc.tile_pool(name="hpool", bufs=3))
        opool = ctx.enter_context(tc.tile_pool(name="opool", bufs=3))
        pxt_pool = ctx.enter_context(tc.tile_pool(name="pxt", bufs=2, space="PSUM"))
        ph_pool = ctx.enter_context(tc.tile_pool(name="ph", bufs=2, space="PSUM"))
        po_pool = ctx.enter_context(tc.tile_pool(name="po", bufs=2, space="PSUM"))

        # Queue the first x loads before weight prep so DMA starts at t=0.
        xbs = []
        for i in range(min(3, N_BLKS)):
            xb = xpool.tile([128, 8, D], F32)
            nc.gpsimd.dma_start(
                xb, x[i * BLK:(i + 1) * BLK, :].rearrange("(p t) d -> p t d", t=8))
            xbs.append(xb)

        ident = const.tile([128, 128], F32)
        make_identity(nc, ident)

        # ---- one-time weight prep (SP HWDGE queue; PSUM scratch from po) ----
        wx_sb = const.tile([H, D], F32)
        nc.sync.dma_start(wx_sb, wh[:, 0:D])
        wxT = const.tile([128, KC, H], F32R)
        for k in range(KC):
            pt = po_pool.tile([128, 8, O], F32, tag="pt")
            nc.tensor.transpose(pt[:, 0, :], wx_sb[:, k * 128:(k + 1) * 128],
                                ident[0:H, 0:H])
            nc.scalar.copy(wxT[:, k, :], pt[:, 0, :])

        wo_sb = const.tile([O, H], F32)
        nc.sync.dma_start(wo_sb, wo)
        woT = const.tile([H, O], BF16)
        pwo = po_pool.tile([128, 8, O], F32, tag="pt")
        nc.tensor.transpose(pwo[0:H, 0, :], wo_sb, ident[0:O, 0:O])
        nc.scalar.copy(woT, pwo[0:H, 0, :])

        bh_sb = const.tile([H, 1], F32)
        nc.sync.dma_start(bh_sb, bh.rearrange("(h one) -> h one", one=1))

        # b_out broadcast to [128, 8, 64] via rank-1 matmul with a ones row
        bo_row = const.tile([1, O], F32)
        nc.sync.dma_start(bo_row, bo.rearrange("(one o) -> one o", one=1))
        ones_row = const.tile([1, 128], F32)
        nc.vector.memset(ones_row, 1.0)
        pbo = po_pool.tile([128, 8, O], F32, tag="pt")
        nc.tensor.matmul(pbo[:, 0, :], lhsT=ones_row, rhs=bo_row,
                         start=True, stop=True)
        bo8 = const.tile([128, 8, O], F32)
        for t in range(8):
            nc.scalar.copy(bo8[:, t, :], pbo[:, 0, :])

        # ---- pipelined main loop ----
        copy_ctr = 0
        for rep in range(repeat):
            for i in range(N_BLKS):
                if rep == 0 and i < len(xbs):
                    xb = xbs[i]
                else:
                    xb = xpool.tile([128, 8, D], F32)
                    nc.gpsimd.dma_start(
                        xb, x[i * BLK:(i + 1) * BLK, :]
                        .rearrange("(p t) d -> p t d", t=8))

                hT = hpool.tile([H, 2, 512], BF16)
                for h in range(2):
                    phT = ph_pool.tile([H, 512], F32)
                    for k in range(KC):
                        ks = slice(k * 128, (k + 1) * 128)
                        pxT = pxt_pool.tile([128, 512], F32)
                        for tt in range(4):
                            nc.tensor.transpose(
                                pxT[:, tt * 128:(tt + 1) * 128],
                                xb[:, 4 * h + tt, ks], ident)
                        xT = xtpool.tile([128, 512], F32R)
                        if copy_ctr % 4 == 0:
                            nc.scalar.copy(xT, pxT)
                        else:
                            nc.vector.tensor_copy(xT, pxT)
                        copy_ctr += 1
                        nc.tensor.matmul(phT, lhsT=wxT[:, k, :], rhs=xT,
                                         start=(k == 0), stop=(k == KC - 1))

                    nc.scalar.activation(hT[:, h, :], phT, TANH,
                                         bias=bh_sb[:, 0:1])

                # mm2: out row 8p+s lives in psum partition p, free slice s
                po_t = po_pool.tile([128, 8, O], F32)
                for s in range(8):
                    h, tt = divmod(s, 4)
                    nc.tensor.matmul(po_t[:, s, :],
                                     lhsT=hT[:, h, tt * 128:(tt + 1) * 128],
                                     rhs=woT, start=True, stop=True)

                nc.vector.tensor_add(po_t, po_t, bo8)
                ob = opool.tile([128, 8, O], F32)
                nc.scalar.activation(ob, po_t, SIGMOID)

                nc.sync.dma_start(
                    out[i * BLK:(i + 1) * BLK, :]
                    .rearrange("(p eight) o -> p eight o", eight=8), ob)

    nc.compile()
    return nc


_NC = None


def _get_nc():
    global _NC
    if _NC is None:
        _NC = build_kernel()
    return _NC


def kernel(x, W_hidden, b_hidden, W_out, b_out):
    x = np.ascontiguousarray(x, dtype=np.float32)
    W_hidden = np.ascontiguousarray(W_hidden, dtype=np.float32)
    b_hidden = np.ascontiguousarray(b_hidden, dtype=np.float32)
    W_out = np.ascontiguousarray(W_out, dtype=np.float32)
    b_out = np.ascontiguousarray(b_out, dtype=np.float32)

    nc = _get_nc()
    shards = np.split(x, N_CORES, axis=0)
    in_maps = [{
        "x": shards[c],
        "w_hidden": W_hidden,
        "b_hidden": b_hidden,
        "w_out": W_out,
        "b_out": b_out,
    } for c in range(N_CORES)]
    res = run_bass_kernel_spmd(nc, in_maps, list(range(N_CORES)))
    return np.concatenate([res.results[c]["out"] for c in range(N_CORES)], axis=0)


if __name__ == "__main__":
    rng = np.random.default_rng(0)
    x = rng.standard_normal((B, D), dtype=np.float32)
    wh = (rng.standard_normal((H, D + O), dtype=np.float32) / np.sqrt(D + O))
    bh_ = rng.standard_normal(H, dtype=np.float32) * 0.01
    wo_ = rng.standard_normal((O, H), dtype=np.float32) / np.sqrt(H)
    bo_ = rng.standard_normal(O, dtype=np.float32) * 0.01
    got = kernel(x=x, W_hidden=wh, b_hidden=bh_, W_out=wo_, b_out=bo_)
    hid = np.tanh(x @ wh[:, :D].T + bh_)
    want = 1.0 / (1.0 + np.exp(-(hid @ wo_.T + bo_)))
    err = np.abs(got - want)
    rel = err.max() / np.abs(want).max()
    print(f"max abs err {err.max():.3e}  rel {rel:.3e}")
